# revision 1
# baseline (speedup 1.0000x reference)
"""Causal attention kernel for Trainium2, sequence-parallel over 8 NeuronCores.

reference:
    q = x @ Wq.T + bq ; k = x @ Wk.T + bk ; v = x @ Wv.T + bv
    scores = q @ k.T / sqrt(D) + mask
    out = softmax(scores, -1) @ v

Sharding: core c owns query rows [c*Q, (c+1)*Q) and the matching mask rows.
x and the weights are replicated; each core works standalone (no collectives).

With zero biases (checked on host) the projections are folded through
associativity:
    scores = x_q @ A @ x.T + mask,   A = Wq.T @ Wk / sqrt(D)   (512x512, tiny)
    out    = (p @ x) @ Wv.T / rowsum(p)
so k and v are never materialized. Softmax needs no max subtraction: scores
are O(1) by construction and masked entries exp to 0 exactly.

Matmuls run in float32r (full-rate fp32 PE mode, ~1e-4 relative accuracy).
"""

import sys
from contextlib import ExitStack, nullcontext

if "/opt/trn_rl_repo" not in sys.path:
    sys.path.insert(0, "/opt/trn_rl_repo")

import numpy as np

import concourse.bass as bass
import concourse.tile as tile
from concourse import bacc, mybir
from concourse.bass_utils import run_bass_kernel_spmd
from concourse.masks import make_identity

F32 = mybir.dt.float32
F32R = mybir.dt.float32r

N, D, NCORES = 8192, 512, 8
P = 128          # partitions
KB = 1024        # key-block size
Q = N // NCORES  # per-core query rows


def build(n=N, d=D, ncores=NCORES, kb=KB, mm_fast=True, use_bias=False,
          reps=1, skip=None, trace_sim=False):
    q_rows = n // ncores
    nqt = q_rows // P            # q-tiles per core
    nb = n // kb                 # key blocks
    tpb = kb // P                # token tiles per block
    dc = d // P                  # d chunks
    inv_sqrt_d = 1.0 / float(np.sqrt(d))
    if skip is None:
        skip = [[False] * nqt for _ in range(nb)]
    first_live = []
    for t in range(nqt):
        live = [b for b in range(nb) if not skip[b][t]]
        assert live, f"q-tile {t} has no live key blocks"
        first_live.append(live[0])
    # Tensors consumed by PE matmuls use float32r (full-rate fp32 mode);
    # the producing instruction must write that dtype (BIR verifier rule).
    MD = F32R if mm_fast else F32

    nc = bacc.Bacc("TRN2", target_bir_lowering=False, debug=False,
                   num_devices=ncores)
    x_d = nc.dram_tensor("x", [n, d], F32, kind="ExternalInput").ap()
    xq_d = nc.dram_tensor("xq", [q_rows, d], F32, kind="ExternalInput").ap()
    mask_d = nc.dram_tensor("mask", [q_rows, n], F32, kind="ExternalInput").ap()
    w_d = {nm: nc.dram_tensor(nm, [d, d], F32, kind="ExternalInput").ap()
           for nm in ("wq", "wk", "wv")}
    b_d = {nm: nc.dram_tensor(nm, [d], F32, kind="ExternalInput").ap()
           for nm in ("bq", "bk", "bv")}
    out_d = nc.dram_tensor("out", [q_rows, d], F32, kind="ExternalOutput").ap()
    assert not use_bias, "zero-bias fast path only; use build_direct for biases"

    # Alternate PSUM->SBUF copies between ACT and DVE to balance engine load.
    flip = [0]

    def copy(out, in_):
        flip[0] ^= 1
        if flip[0]:
            nc.scalar.copy(out=out, in_=in_)
        else:
            nc.vector.tensor_copy(out=out, in_=in_)

    with tile.TileContext(nc, trace_sim=trace_sim) as tc, ExitStack() as st:
        consts = st.enter_context(tc.tile_pool(name="consts", bufs=1))
        wts = st.enter_context(tc.tile_pool(name="wts", bufs=1))
        wnat_p = st.enter_context(tc.tile_pool(name="wnat", bufs=1))
        qt_p = st.enter_context(tc.tile_pool(name="qt", bufs=1))
        xtb_p = st.enter_context(tc.tile_pool(name="xtb", bufs=2))
        xs_p = st.enter_context(tc.tile_pool(name="xs", bufs=3))
        xnb_p = st.enter_context(tc.tile_pool(name="xnb", bufs=2))
        mask_p = st.enter_context(tc.tile_pool(name="maskp", bufs=2))
        p_p = st.enter_context(tc.tile_pool(name="pp", bufs=2))
        pt_p = st.enter_context(tc.tile_pool(name="ptp", bufs=2))
        out_p = st.enter_context(tc.tile_pool(name="outp", bufs=2))
        ps_tr = st.enter_context(tc.tile_pool(name="ps_tr", bufs=2, space="PSUM"))
        ps_s = st.enter_context(tc.tile_pool(name="ps_s", bufs=4, space="PSUM"))
        ps_pv = st.enter_context(tc.tile_pool(name="ps_pv", bufs=2, space="PSUM"))

        loop = tc.For_i(0, reps, 1) if reps > 1 else nullcontext()
        with loop:
            ident = consts.tile([P, P], F32, tag="ident")
            make_identity(nc, ident)
            ident_r = consts.tile([P, P], MD, tag="ident_r")
            nc.vector.tensor_copy(out=ident_r, in_=ident)

            # ---- wvT[p, c, dout] = Wv[dout, c*P+p] ----
            wvn = wnat_p.tile([P, dc, d], F32, tag="wnat")
            nc.gpsimd.dma_start(out=wvn,
                                in_=w_d["wv"].rearrange("(j p) d -> p j d", p=P))
            wvT = wts.tile([P, dc, d], MD, tag="wvT")
            for i in range(dc):
                ps = ps_tr.tile([P, 512], F32, tag="ps_tr")
                for j in range(dc):
                    nc.tensor.transpose(ps[:, j * P:(j + 1) * P],
                                        wvn[:, j, i * P:(i + 1) * P], ident)
                copy(wvT[:, i, :].rearrange("p (j f) -> p j f", f=P),
                     ps.rearrange("p (j f) -> p j f", f=P))

            # ---- A = Wq.T @ Wk * inv_sqrt_d,  A_sb[p, i, j] = A[i*P+p, j] ----
            wqk = {}
            for nm in ("wq", "wk"):
                wn = wnat_p.tile([P, dc, d], F32, tag="wnat")
                nc.gpsimd.dma_start(
                    out=wn, in_=w_d[nm].rearrange("(j p) d -> p j d", p=P))
                wqk[nm] = wts.tile([P, dc, d], MD, tag=f"{nm}n", name=f"{nm}n")
                copy(wqk[nm], wn)
            A_sb = wts.tile([P, dc, d], MD, tag="A_sb")
            for i in range(dc):
                ps = ps_tr.tile([P, 512], F32, tag="ps_tr")
                for m in range(dc):
                    nc.tensor.matmul(ps, wqk["wq"][:, m, i * P:(i + 1) * P],
                                     wqk["wk"][:, m, :],
                                     start=(m == 0), stop=(m == dc - 1))
                nc.scalar.mul(out=A_sb[:, i, :], in_=ps, mul=inv_sqrt_d)

            # ---- xqT then yT = A.T @ xqT  (plays the role of scaled qT) ----
            xqT = xtb_p.tile([P, dc, max(q_rows, kb)], MD, tag="xtb")
            for t in range(nqt):
                xt = xs_p.tile([P, dc, d], F32, tag="xs")
                nc.gpsimd.dma_start(out=xt[:, 0, :],
                                    in_=xq_d[t * P:(t + 1) * P, :])
                ps = ps_tr.tile([P, 512], F32, tag="ps_tr")
                for i in range(dc):
                    nc.tensor.transpose(ps[:, i * P:(i + 1) * P],
                                        xt[:, 0, i * P:(i + 1) * P], ident)
                copy(xqT[:, :, t * P:(t + 1) * P],
                     ps.rearrange("p (i f) -> p i f", f=P))

            yT = qt_p.tile([P, dc, q_rows], MD, tag="yT")
            for m in range(dc):
                for n0 in range(0, q_rows, 512):
                    nn = min(512, q_rows - n0)
                    ps = ps_tr.tile([P, 512], F32, tag="ps_tr")
                    for c in range(dc):
                        nc.tensor.matmul(ps[:, :nn],
                                         A_sb[:, c, m * P:(m + 1) * P],
                                         xqT[:, c, n0:n0 + nn],
                                         start=(c == 0), stop=(c == dc - 1))
                    copy(yT[:, m, n0:n0 + nn], ps[:, :nn])

            # per-(q-tile, block, half) exp row-sums; reduced at the end
            l_all = consts.tile([P, nqt, 2 * nb], F32, tag="lall")
            nc.vector.memset(l_all, 0.0)
            linv = consts.tile([P, nqt], F32, tag="linv")
            zacc = consts.tile([P, nqt, d], F32, tag="zacc")

            # ---- main loop over key blocks ----
            for b in range(nb):
                if all(skip[b][t] for t in range(nqt)):
                    continue
                # stream x rows, transpose to xTB [p, c, kb]; keep natural
                # copy xNB [p, t, d] (f32r) as the p@x rhs
                xTB = xtb_p.tile([P, dc, max(q_rows, kb)], MD, tag="xtb")
                xNB = xnb_p.tile([P, tpb, d], MD, tag="xnb")
                for h in range(0, tpb, dc):
                    nh = min(dc, tpb - h)
                    xh = xs_p.tile([P, dc, d], F32, tag="xs")
                    nc.gpsimd.dma_start(
                        out=xh[:, :nh, :],
                        in_=x_d[b * kb + h * P: b * kb + (h + nh) * P, :]
                            .rearrange("(t p) d -> p t d", p=P))
                    copy(xNB[:, h:h + nh, :], xh[:, :nh, :])
                    for t in range(nh):
                        ps = ps_tr.tile([P, 512], F32, tag="ps_tr")
                        for i in range(dc):
                            nc.tensor.transpose(ps[:, i * P:(i + 1) * P],
                                                xh[:, t, i * P:(i + 1) * P],
                                                ident)
                        copy(xTB[:, :, (h + t) * P:(h + t + 1) * P],
                             ps.rearrange("p (i f) -> p i f", f=P))

                # attention for each q-tile against this block
                for t in range(nqt):
                    if skip[b][t]:
                        continue
                    mk = mask_p.tile([P, kb], F32, tag="maskp")
                    nc.gpsimd.dma_start(
                        out=mk,
                        in_=mask_d[t * P:(t + 1) * P, b * kb:(b + 1) * kb])
                    pt = p_p.tile([P, kb], MD, tag="pp")
                    for h0 in range(0, kb, 512):
                        ps = ps_s.tile([P, 512], F32, tag="ps_s")
                        for c in range(dc):
                            nc.tensor.matmul(ps,
                                             yT[:, c, t * P:(t + 1) * P],
                                             xTB[:, c, h0:h0 + 512],
                                             start=(c == 0), stop=(c == dc - 1))
                        nc.vector.tensor_add(out=ps, in0=ps,
                                             in1=mk[:, h0:h0 + 512])
                        col = 2 * b + h0 // 512
                        nc.scalar.activation(
                            out=pt[:, h0:h0 + 512], in_=ps,
                            func=mybir.ActivationFunctionType.Exp,
                            accum_out=l_all[:, t, col:col + 1])
                    # transpose p -> pT [p(key), t, q]
                    pT = pt_p.tile([P, tpb, P], MD, tag="ptp")
                    for h in range(0, tpb, dc):
                        ps = ps_tr.tile([P, 512], MD, tag="ps_tr")
                        for j in range(dc):
                            nc.tensor.transpose(
                                ps[:, j * P:(j + 1) * P],
                                pt[:, (h + j) * P:(h + j + 1) * P], ident_r)
                        copy(pT[:, h:h + dc, :],
                             ps.rearrange("p (j f) -> p j f", f=P))
                    # z += p @ x_block
                    ps = ps_pv.tile([P, d], F32, tag="ps_pv")
                    for kbi in range(tpb):
                        nc.tensor.matmul(ps, pT[:, kbi, :], xNB[:, kbi, :],
                                         start=(kbi == 0), stop=(kbi == tpb - 1))
                    if b == first_live[t]:
                        copy(zacc[:, t, :], ps)
                    else:
                        nc.vector.tensor_add(out=zacc[:, t, :],
                                             in0=zacc[:, t, :], in1=ps)

            # ---- finalize: out = (z / l) @ Wv.T ----
            for t in range(nqt):
                lsum = out_p.tile([P, 1], F32, tag="lsum")
                nc.vector.reduce_sum(lsum, l_all[:, t, :],
                                     axis=mybir.AxisListType.X)
                nc.vector.reciprocal(linv[:, t:t + 1], lsum)
                zn = out_p.tile([P, d], F32, tag="zn")
                nc.vector.tensor_scalar_mul(out=zn, in0=zacc[:, t, :],
                                            scalar1=linv[:, t:t + 1])
                ps = ps_tr.tile([P, 512], F32, tag="ps_tr")
                for i in range(dc):
                    nc.tensor.transpose(ps[:, i * P:(i + 1) * P],
                                        zn[:, i * P:(i + 1) * P], ident)
                znT = out_p.tile([P, dc, P], MD, tag="znT")
                copy(znT, ps.rearrange("p (i f) -> p i f", f=P))
                ops = ps_tr.tile([P, 512], F32, tag="ps_tr")
                for c in range(dc):
                    nc.tensor.matmul(ops, znT[:, c, :], wvT[:, c, :],
                                     start=(c == 0), stop=(c == dc - 1))
                ot = out_p.tile([P, d], F32, tag="outp")
                copy(ot, ops)
                nc.gpsimd.dma_start(out=out_d[t * P:(t + 1) * P, :], in_=ot)

    nc.compile()
    return nc


def core_rows(n, ncores, c):
    """Cyclic-by-128-row-tile sharding: core c owns global tiles c, c+ncores, ..."""
    nt_global = n // P
    tiles = list(range(c, nt_global, ncores))
    return np.concatenate([np.arange(g * P, (g + 1) * P) for g in tiles])


def prepare_in_maps(x, mask, Wq, bq, Wk, bk, Wv, bv, n=None, ncores=NCORES,
                    kb=KB):
    """Cyclic q-tile sharding + per-(block, tile) full-mask skip table.

    A (q-tile, key-block) pair is skipped only when EVERY core's mask block
    at that position is entirely <= -1e8: exp(scores + mask) underflows to
    exactly 0.0 there, so skipping is bit-exact. With a causal mask the
    cyclic assignment makes each core skip the same ~44% of pairs.
    """
    x = np.asarray(x); mask = np.asarray(mask)
    if n is None:
        n = x.shape[0]
    q_rows = n // ncores
    nqt = q_rows // P
    nb = n // kb
    f = np.ascontiguousarray
    rows = [core_rows(n, ncores, c) for c in range(ncores)]
    # skip[b][t] must hold for every core (the SPMD program is shared)
    skip = [[True] * nqt for _ in range(nb)]
    for c in range(ncores):
        m = mask[rows[c]]
        blk = m.reshape(nqt, P, nb, kb).max(axis=(1, 3))  # [nqt, nb]
        for b in range(nb):
            for t in range(nqt):
                if blk[t, b] > -1e8:
                    skip[b][t] = False
    in_maps = [
        {
            "x": f(x.astype(np.float32)),
            "xq": f(x[rows[c]].astype(np.float32)),
            "mask": f(mask[rows[c]].astype(np.float32)),
            "wq": f(np.asarray(Wq).astype(np.float32)),
            "bq": f(np.asarray(bq).astype(np.float32)),
            "wk": f(np.asarray(Wk).astype(np.float32)),
            "bk": f(np.asarray(bk).astype(np.float32)),
            "wv": f(np.asarray(Wv).astype(np.float32)),
            "bv": f(np.asarray(bv).astype(np.float32)),
        }
        for c in range(ncores)
    ]
    meta = {"skip": skip, "rows": rows}
    return in_maps, meta


def make_in_maps(x, mask, Wq, bq, Wk, bk, Wv, bv, ncores=NCORES, kb=KB):
    in_maps, _ = prepare_in_maps(x, mask, Wq, bq, Wk, bk, Wv, bv,
                                 ncores=ncores, kb=kb)
    return in_maps


_CACHED = {}


def kernel(x, mask, Wq, bq, Wk, bk, Wv, bv):
    x = np.asarray(x)
    in_maps, meta = prepare_in_maps(x, mask, Wq, bq, Wk, bk, Wv, bv)
    key = bytes(bytearray(b for row in meta["skip"] for b in row))
    if _CACHED.get("key") != key:
        _CACHED["nc"] = build(skip=meta["skip"])
        _CACHED["key"] = key
    nc = _CACHED["nc"]
    res = run_bass_kernel_spmd(nc, in_maps, list(range(NCORES)))
    out = np.empty((x.shape[0], x.shape[1]), np.float32)
    for c in range(NCORES):
        out[meta["rows"][c]] = res.results[c]["out"]
    return out



# revision 2
# speedup vs baseline: 1.3973x; 1.3973x over previous
"""Causal attention for Trainium2, sequence-parallel over 8 NeuronCores. v3.

Differences vs kernel2 (transposed-scores):
  * Scores computed in NATURAL layout [q, keys]: 4 chained 512-col matmuls
    per 4-key-tile group (minimal PE instruction count).
  * Row-sums l come FREE from the Exp activation's accum_out (sums along
    the free/key axis in natural layout) -- no ones-matmuls.
  * Causality on the last 8 key tiles enforced with an ADDITIVE bf16 mask
    (0 / -32768) on the PSUM scores before exp (exp underflows to 0).
  * p is transposed on PE (4 identity-matmuls per group) into pT, which
    feeds the same z accumulation as kernel2.
  * Finalize is stage-split across the next q-tile's groups.
"""

import sys
from contextlib import ExitStack, nullcontext

if "/opt/trn_rl_repo" not in sys.path:
    sys.path.insert(0, "/opt/trn_rl_repo")

import numpy as np
import ml_dtypes

import concourse.bass as bass
import concourse.tile as tile
from concourse import bacc, mybir
from concourse.bass_utils import run_bass_kernel_spmd
from concourse.masks import make_identity

F32 = mybir.dt.float32
BF16 = mybir.dt.bfloat16
NPBF = ml_dtypes.bfloat16

N, D, NCORES = 8192, 512, 8
P = 128
NT = 8                 # q-tiles per core
DC = D // P            # 4 d-chunks
KT = N // P            # 64 key tiles total
MASKVAL = -32768.0     # additive causal mask; exp() underflows to exactly 0


def build(reps=1, trace_sim=False, stage="full"):
    q_rows = NT * P
    nc = bacc.Bacc("TRN2", target_bir_lowering=False, debug=False,
                   num_devices=NCORES)
    xn_d = nc.dram_tensor("xn", [N, D], BF16, kind="ExternalInput").ap()
    xt_d = nc.dram_tensor("xt", [D, N], BF16, kind="ExternalInput").ap()
    xqt_d = nc.dram_tensor("xqt", [D, q_rows], BF16, kind="ExternalInput").ap()
    a_d = nc.dram_tensor("amat", [D, D], BF16, kind="ExternalInput").ap()
    wvt_d = nc.dram_tensor("wvt", [D, D], BF16, kind="ExternalInput").ap()
    v_d = nc.dram_tensor("vvec", [P, DC], F32, kind="ExternalInput").ap()
    mq_d = nc.dram_tensor("maskq", [P, NT * P], BF16, kind="ExternalInput").ap()
    out_d = nc.dram_tensor("out", [q_rows, D], F32, kind="ExternalOutput").ap()

    with tile.TileContext(nc, trace_sim=trace_sim) as tc, ExitStack() as st:
        consts = st.enter_context(tc.tile_pool(name="consts", bufs=1))
        big = st.enter_context(tc.tile_pool(name="big", bufs=1))
        la_p = st.enter_context(tc.tile_pool(name="lap", bufs=2))
        pn_p = st.enter_context(tc.tile_pool(name="pnp", bufs=3))
        pt_p = st.enter_context(tc.tile_pool(name="ptp", bufs=3))
        zn_p = st.enter_context(tc.tile_pool(name="znp", bufs=2))
        out_p = st.enter_context(tc.tile_pool(name="outp", bufs=2))
        ps_s = st.enter_context(tc.tile_pool(name="ps_s", bufs=2, space="PSUM"))
        ps_t = st.enter_context(tc.tile_pool(name="ps_t", bufs=2, space="PSUM"))
        ps_z = st.enter_context(tc.tile_pool(name="ps_z", bufs=2, space="PSUM"))
        ps_fo = st.enter_context(tc.tile_pool(name="ps_fo", bufs=2, space="PSUM"))

        loop = tc.For_i(0, reps, 1) if reps > 1 else nullcontext()
        with loop:
            ident = consts.tile([P, P], F32, tag="ident")
            make_identity(nc, ident)
            ident_b = consts.tile([P, P], BF16, tag="ident_b")
            nc.vector.tensor_copy(out=ident_b, in_=ident)

            a_sb = consts.tile([P, DC, D], BF16, tag="a_sb")
            nc.gpsimd.dma_start(out=a_sb,
                                in_=a_d.rearrange("(c p) d -> p c d", p=P))
            xqt_sb = consts.tile([P, DC, q_rows], BF16, tag="xqt")
            nc.gpsimd.dma_start(out=xqt_sb,
                                in_=xqt_d.rearrange("(c p) n -> p c n", p=P))
            v_sb = consts.tile([P, DC], F32, tag="v_sb")
            nc.gpsimd.dma_start(out=v_sb, in_=v_d)
            mq_sb = consts.tile([P, NT * P], BF16, tag="mq")
            nc.gpsimd.dma_start(out=mq_sb, in_=mq_d)

            # x resident in both layouts; 16 chunks, xt slice before xn slice
            # (scores consume xt earlier than z consumes xn).
            xn_sb = big.tile([P, KT, D], BF16, tag="xn")
            xt_sb = big.tile([P, DC, N], BF16, tag="xt")
            wvt_sb = consts.tile([P, DC, D], BF16, tag="wvt")
            for b in range(16):
                r0 = b * 512
                nc.gpsimd.dma_start(
                    out=xt_sb[:, :, r0:r0 + 512],
                    in_=xt_d[:, r0:r0 + 512].rearrange("(c p) n -> p c n", p=P))
                nc.gpsimd.dma_start(
                    out=xn_sb[:, 4 * b:4 * b + 4, :],
                    in_=xn_d[r0:r0 + 512, :].rearrange("(t p) d -> p t d", p=P))
                if b == 0:
                    nc.gpsimd.dma_start(
                        out=wvt_sb,
                        in_=wvt_d.rearrange("(c p) d -> p c d", p=P))

            # ---- yT[dch*P+p, q] = (A.T @ xq.T)[d, q] + v[d] ----
            yT = consts.tile([P, DC, q_rows], BF16, tag="yT")
            for dch in range(DC):
                for qh in range(q_rows // 512):
                    ps = ps_fo.tile([P, 512], F32, tag="ps_fo")
                    for c in range(DC):
                        nc.tensor.matmul(ps,
                                         a_sb[:, c, dch * P:(dch + 1) * P],
                                         xqt_sb[:, c, qh * 512:(qh + 1) * 512],
                                         start=(c == 0), stop=(c == DC - 1))
                    nc.vector.tensor_scalar_add(
                        out=yT[:, dch, qh * 512:(qh + 1) * 512], in0=ps,
                        scalar1=v_sb[:, dch:dch + 1])

            linv = consts.tile([P, NT], F32, tag="linv")

            def fin_a(t, psz, la, G):
                lsum = out_p.tile([P, 1], F32, tag="lsum")
                nc.vector.reduce_sum(lsum, la[:, :G], axis=mybir.AxisListType.X)
                nc.vector.reciprocal(linv[:, t:t + 1], lsum)
                zn = zn_p.tile([P, D], F32, tag="zn")
                nc.vector.tensor_copy(out=zn, in_=psz)
                return zn

            def fin_b(zn):
                psf = ps_fo.tile([P, 512], F32, tag="ps_fo")
                for ch in range(DC):
                    nc.tensor.transpose(psf[:, ch * P:(ch + 1) * P],
                                        zn[:, ch * P:(ch + 1) * P], ident)
                return psf

            def fin_c(psf):
                znT = zn_p.tile([P, D], BF16, tag="znT")
                nc.vector.tensor_copy(out=znT, in_=psf)
                return znT

            def fin_d(t, znT):
                pso = ps_fo.tile([P, 512], F32, tag="ps_fo")
                for ch in range(DC):
                    nc.tensor.matmul(pso, znT[:, ch * P:(ch + 1) * P],
                                     wvt_sb[:, ch, :],
                                     start=(ch == 0), stop=(ch == DC - 1))
                ot = out_p.tile([P, D], F32, tag="outp")
                nc.vector.tensor_scalar_mul(out=ot, in0=pso,
                                            scalar1=linv[:, t:t + 1])
                nc.gpsimd.dma_start(out=out_d[t * P:(t + 1) * P, :], in_=ot)

            # pending finalize state machine: list of (stage_idx, t, payload)
            pending = []

            def pump_fin():
                if not pending:
                    return
                st_i, ft, payload = pending[0]
                if st_i == 0:
                    pending[0] = (1, ft, fin_a(ft, *payload))
                elif st_i == 1:
                    pending[0] = (2, ft, fin_b(payload))
                elif st_i == 2:
                    pending[0] = (3, ft, fin_c(payload))
                else:
                    fin_d(ft, payload)
                    pending.pop(0)

            for t in range(NT):
                Kt = 8 * (t + 1)
                G = Kt // 4
                psz = ps_z.tile([P, D], F32, tag="ps_z")
                la = la_p.tile([P, 16], F32, tag="la")
                tq = []   # (i, pt_nat) awaiting transpose
                zq = []   # (i, pT_sb) awaiting z-matmuls
                fin_budget = 4
                for i in range(G):
                    pss = ps_s.tile([P, 512], F32, tag="ps_s")
                    for c in range(DC):
                        nc.tensor.matmul(pss,
                                         yT[:, c, t * P:(t + 1) * P],
                                         xt_sb[:, c, 4 * i * P:(4 * i + 4) * P],
                                         start=(c == 0), stop=(c == DC - 1))
                    if i >= G - 2:
                        mh = i - (G - 2)
                        nc.vector.tensor_add(
                            out=pss, in0=pss,
                            in1=mq_sb[:, mh * 512:(mh + 1) * 512])
                    if stage == "s":
                        continue
                    pn = pn_p.tile([P, 512], BF16, tag="pn")
                    nc.scalar.activation(out=pn, in_=pss,
                                         func=mybir.ActivationFunctionType.Exp,
                                         accum_out=la[:, i:i + 1])
                    if stage == "se":
                        continue
                    tq.append((i, pn))
                    # lag-1: transpose group i-1 while scores of i stream
                    if len(tq) > 1:
                        ti, tpn = tq.pop(0)
                        pst = ps_t.tile([P, 512], BF16, tag="ps_t")
                        for j in range(4):
                            nc.tensor.transpose(pst[:, j * P:(j + 1) * P],
                                                tpn[:, j * P:(j + 1) * P],
                                                ident_b)
                        ptb = pt_p.tile([P, 512], BF16, tag="pt")
                        nc.vector.tensor_copy(out=ptb, in_=pst)
                        zq.append((ti, ptb))
                    if len(zq) > 1:
                        zi, zpt = zq.pop(0)
                        for j in range(4):
                            kt = 4 * zi + j
                            nc.tensor.matmul(psz, zpt[:, j * P:(j + 1) * P],
                                             xn_sb[:, kt, :],
                                             start=(kt == 0),
                                             stop=(kt == Kt - 1))
                    if fin_budget > 0 and i >= 1:
                        pump_fin()
                        fin_budget -= 1
                if stage in ("s", "se"):
                    continue
                # drain
                while tq:
                    ti, tpn = tq.pop(0)
                    pst = ps_t.tile([P, 512], BF16, tag="ps_t")
                    for j in range(4):
                        nc.tensor.transpose(pst[:, j * P:(j + 1) * P],
                                            tpn[:, j * P:(j + 1) * P], ident_b)
                    ptb = pt_p.tile([P, 512], BF16, tag="pt")
                    nc.vector.tensor_copy(out=ptb, in_=pst)
                    zq.append((ti, ptb))
                while zq:
                    zi, zpt = zq.pop(0)
                    for j in range(4):
                        kt = 4 * zi + j
                        nc.tensor.matmul(psz, zpt[:, j * P:(j + 1) * P],
                                         xn_sb[:, kt, :],
                                         start=(kt == 0), stop=(kt == Kt - 1))
                if stage == "full":
                    pending.append((0, t, (psz, la, G)))
            if stage == "full":
                while pending:
                    pump_fin()

    nc.compile()
    return nc


def core_rows(c):
    tiles = list(range(c, KT, NCORES))
    return np.concatenate([np.arange(g * P, (g + 1) * P) for g in tiles])


def _check_causal_mask(mask):
    m = np.asarray(mask)
    assert m.shape == (N, N), f"mask shape {m.shape}"
    rng = np.random.default_rng(0)
    rows = rng.choice(N, size=64, replace=False)
    cols = np.arange(N)
    sub = m[rows]
    expect = np.where(cols[None, :] <= rows[:, None], 0.0, -1e9).astype(np.float32)
    if not np.array_equal(sub, expect):
        raise ValueError("mask is not the expected causal mask; "
                         "this kernel hardcodes causal structure")


def prepare_in_maps(x, mask, Wq, bq, Wk, bk, Wv, bv):
    x = np.asarray(x, dtype=np.float32)
    _check_causal_mask(mask)
    inv_sqrt_d = 1.0 / np.sqrt(D)
    A = (np.asarray(Wq).T.astype(np.float64) @ np.asarray(Wk).astype(np.float64)
         * inv_sqrt_d).astype(np.float32)
    wvT = np.ascontiguousarray(np.asarray(Wv).T)
    vvec = (np.asarray(Wk).T @ np.asarray(bq) * inv_sqrt_d).astype(np.float32)
    vvec = np.ascontiguousarray(vvec.reshape(DC, P).T)  # [P, DC]
    xn_b = x.astype(NPBF)
    xt_b = np.ascontiguousarray(x.T).astype(NPBF)
    a_b = A.astype(NPBF)
    wvt_b = wvT.astype(NPBF)

    qp = np.arange(P)[:, None, None]
    kl = np.arange(NT)[None, :, None]
    kp = np.arange(P)[None, None, :]
    rows = [core_rows(c) for c in range(NCORES)]
    in_maps = []
    for c in range(NCORES):
        live = (kl * P + kp <= c * P + qp)           # [qp, kl, kp]
        mq = np.where(live, 0.0, MASKVAL).astype(NPBF).reshape(P, NT * P)
        xqt = np.ascontiguousarray(x[rows[c]].T).astype(NPBF)
        in_maps.append({
            "xn": xn_b, "xt": xt_b, "xqt": xqt, "amat": a_b,
            "wvt": wvt_b, "vvec": vvec,
            "maskq": np.ascontiguousarray(mq),
        })
    meta = {"rows": rows, "bv": np.asarray(bv, dtype=np.float32)}
    return in_maps, meta


_CACHED = {}


def kernel(x, mask, Wq, bq, Wk, bk, Wv, bv):
    x = np.asarray(x)
    in_maps, meta = prepare_in_maps(x, mask, Wq, bq, Wk, bk, Wv, bv)
    if "nc" not in _CACHED:
        _CACHED["nc"] = build()
    nc = _CACHED["nc"]
    res = run_bass_kernel_spmd(nc, in_maps, list(range(NCORES)))
    out = np.empty((N, D), np.float32)
    for c in range(NCORES):
        out[meta["rows"][c]] = res.results[c]["out"]
    out += meta["bv"][None, :]
    return out
